# revision 13
# baseline (speedup 1.0000x reference)
"""Trainium2 Bass kernel for nn_AttentionSimple (sparse_attention, 8 cores).

Reference (per batch row b):
    e      = embeddings[k[b]]              # [S, E] gather
    scores = q[b] . e[s]                   # [S]
    attn   = softmax(scores); ctx = sum_s attn[s] * e[s]
    out    = ctx @ W.T + b                 # [B, 2]

Algorithm: count-weighted vocab-space softmax — no per-token gathers.
Scores depend on s only through v = k[b, s], so group softmax terms by
vocabulary id:
    c[b, v]  = |{s : k[b, s] = v}|         (histogram of k, built on host)
    l[b, v]  = q[b] . embeddings[v]        (dense PE matmul)
    A        = c * exp(l - 30)             (global bias keeps exp in fp16;
                                            the ratio is shift-invariant)
    out[b]   = (sum_v A[b,v] * EW[v]) / (sum_v A[b,v])
    with EW  = embeddings @ W.T + b        (parameter prepacking, host)

Sharding: padded vocabulary 51200 = 400 chunks of 128, 50 chunks/core.
Each core handles all 128 batch rows for its slice; host sums the 8
partial numerators/denominators and divides.

v2 pipeline (per core), all wire data fp16 except uint8 counts:
  - et: embedding pairs packed at partition rows 0:50 / 50:100 (no
    zero-pad rows on the wire), [100, 25*128] fp16 — half the f32 bytes.
  - mm1: per pair, ps[128v, 256] = et_pair.T @ qw (qw = block-diag
    [qT|0; 0|qT] fp16, [100, 256]); 4 pairs fill a [128, 1024] 2-bank
    PSUM block (2 quads).
  - ACT: le = exp(ps - 30) fused PSUM->SBUF, fp16 out, one 1024-col
    instruction per block (amortizes the ~250ns fixed ACT overhead).
  - DVE: le *= counts (uint8, exact; max count 5) in place, 1024 cols.
  - mm2: acc[9, 512] += st9_quad.T @ le_quad (fp16, f32 PSUM accum);
    st9 = [EW_c0..EW_c3 | ones] per quad; 13 accumulating matmuls.
  - 50 chunks = 12 full quads + 1 half quad (chunks 48,49); the half
    block's unused PSUM cols never feed mm2 (rhs is narrowed to 256).
  - Warm-up matmuls ramp the PE p-state while the first DMAs land.
  - Inputs live in single resident SBUF tiles; DMAs are column-sliced
    so compute only waits on the slice it reads (subtile deps).  et
    slices ride the Sync queue, qw/st/ct ride the Scalar queue, so
    issue cost is split across two queues.
  - Output: acc PSUM is DMA'd straight to DRAM (no SBUF copy).
"""

import numpy as np

BATCH, SEQ, EMB, VOCAB, OUT = 128, 8192, 50, 50000, 2
N_CORES = 8
CSH = 50                         # vocab chunks per core
NCHUNK = CSH * N_CORES           # 400
VPAD = NCHUNK * 128              # 51200
VSH = CSH * 128                  # 6400
NPAIR = CSH // 2                 # 25
NQUAD = 13                       # 12 full + 1 half
NBLOCK = 7                       # 6 full (4 pairs) + 1 micro (1 pair)
EXP_BIAS = -30.0
NWARM = 3

_CACHE = {}


def _build_nc():
    from contextlib import ExitStack

    import concourse.mybir as mybir
    import concourse.tile as tile
    from concourse import bacc

    f32 = mybir.dt.float32
    f16 = mybir.dt.float16
    u8 = mybir.dt.uint8
    nc = bacc.Bacc("TRN2", target_bir_lowering=False, debug=False,
                   num_devices=N_CORES)

    et_d = nc.dram_tensor("et", [100, NPAIR * 128], f16, kind="ExternalInput")
    qw_d = nc.dram_tensor("qw", [100, 256], f16, kind="ExternalInput")
    st_d = nc.dram_tensor("st", [128, NQUAD * 9], f16, kind="ExternalInput")
    ct_d = nc.dram_tensor("ct", [128, VSH], u8, kind="ExternalInput")
    o_d = nc.dram_tensor("o", [18, 512], f32, kind="ExternalOutput")

    with tile.TileContext(nc) as tc, ExitStack() as ctx:
        const_p = ctx.enter_context(tc.tile_pool(name="const", bufs=1))
        ps_p = ctx.enter_context(tc.tile_pool(name="ps", bufs=3, space="PSUM"))
        acc_p = ctx.enter_context(tc.tile_pool(name="acc", bufs=1,
                                               space="PSUM"))
        le_p = ctx.enter_context(tc.tile_pool(name="le", bufs=3))

        # Two accumulators: accA (quads 0-5) finalizes mid-kernel so its
        # copy + output DMA overlap the remaining blocks; accB takes the
        # rest.  Warm-up matmuls write into accB's bank (start=True on the
        # first real accB matmul resets it), keeping PSUM within 8 banks.
        accA = acc_p.tile([9, 512], f32, tag="accA")
        accBf = acc_p.tile([128, 512], f32, tag="accB")
        accB = accBf[0:9, :]

        # PE warm-up: matmuls on a zeroed tile while input DMAs land, so
        # the PE p-state ramp starts as early as possible.  The memsets
        # ride the (otherwise idle) GpSimd queue so the warm-ups can start
        # right after the PE passes the preamble.
        wtile = const_p.tile([128, 512], f16)
        nc.gpsimd.memset(wtile[:], 0.0)
        bias_sb = const_p.tile([128, 1], f32)
        nc.gpsimd.memset(bias_sb[:], EXP_BIAS)
        for _ in range(NWARM):
            nc.tensor.matmul(accBf[:], lhsT=wtile[:, 0:128], rhs=wtile[:],
                             start=True, stop=True, skip_group_check=True)

        # Resident input tiles.  The critical first et slice goes out
        # first and alone on the Sync queue; counts ride the Scalar queue,
        # whose ACT-table load naturally delays them past the et head.
        qw_sb = const_p.tile([100, 256], f16)
        st_sb = const_p.tile([128, NQUAD * 9], f16)
        et_sb = const_p.tile([100, NPAIR * 128], f16)
        ct_sb = const_p.tile([128, VSH], u8)
        nc.sync.dma_start(et_sb[:, 0:512], et_d.ap()[:, 0:512])
        nc.sync.dma_start(qw_sb[:], qw_d.ap())
        nc.sync.dma_start(et_sb[:, 512:2048], et_d.ap()[:, 512:2048])
        nc.sync.dma_start(et_sb[:, 2048:3200], et_d.ap()[:, 2048:3200])
        nc.scalar.dma_start(ct_sb[:, 0:2048], ct_d.ap()[:, 0:2048])
        nc.scalar.dma_start(st_sb[:], st_d.ap())
        nc.scalar.dma_start(ct_sb[:, 2048:4096], ct_d.ap()[:, 2048:4096])
        nc.scalar.dma_start(ct_sb[:, 4096:VSH], ct_d.ap()[:, 4096:VSH])

        osbA = const_p.tile([9, 512], f32)
        osbB = const_p.tile([9, 512], f32)

        # Block structure: small head block (short pipeline fill) and
        # small tail blocks (short drain).  (pair0, quad0, npair):
        BLOCKS = [(0, 0, 2), (2, 1, 4), (6, 3, 4), (10, 5, 4),
                  (14, 7, 4), (18, 9, 4), (22, 11, 2), (24, 12, 1)]
        for blk, (pair0, quad0, npair) in enumerate(BLOCKS):
            ncol = 256 * npair
            ps = ps_p.tile([128, 1024], f32, tag="ps")
            for lp in range(npair):
                pair = pair0 + lp
                nc.tensor.matmul(
                    ps[:, lp * 256:(lp + 1) * 256],
                    lhsT=et_sb[:, pair * 128:(pair + 1) * 128],
                    rhs=qw_sb[:],
                    start=True, stop=True,
                )
            le = le_p.tile([128, 1024], f16, tag="le")
            nc.scalar.activation(le[:, 0:ncol], ps[:, 0:ncol],
                                 mybir.ActivationFunctionType.Exp,
                                 bias=bias_sb[:])
            nc.vector.tensor_mul(
                le[:, 0:ncol], le[:, 0:ncol],
                ct_sb[:, pair0 * 256:pair0 * 256 + ncol])
            nquad = (npair + 1) // 2
            for lq in range(nquad):
                quad = quad0 + lq
                qcol = min(512, ncol - lq * 512)
                acc = accA if quad < 6 else accB
                nc.tensor.matmul(
                    acc if qcol == 512 else acc[:, 0:qcol],
                    lhsT=st_sb[:, quad * 9:(quad + 1) * 9],
                    rhs=le[:, lq * 512:lq * 512 + qcol],
                    start=(quad in (0, 6)), stop=(quad in (5, NQUAD - 1)),
                    skip_group_check=True,
                )
            if blk == 3:
                # accA finalized (quads 0-5): ship it while blocks 4-7 run.
                nc.scalar.copy(osbA[:], accA)
                nc.sync.dma_start(o_d.ap()[0:9, :], osbA[:])

        nc.scalar.copy(osbB[:], accB)
        nc.scalar.dma_start(o_d.ap()[9:18, :], osbB[:])

    nc.finalize()
    return nc


def _prep_inputs(q, k, embeddings, W, b):
    q = np.ascontiguousarray(q, dtype=np.float32)
    emb = np.ascontiguousarray(embeddings, dtype=np.float32)
    W = np.ascontiguousarray(W, dtype=np.float32)
    b = np.ascontiguousarray(b, dtype=np.float32)
    k = np.asarray(k)

    embT = np.zeros((EMB, VPAD), np.float32)
    embT[:, :VOCAB] = emb.T

    # mm1 moving operand: block-diagonal [qT | 0; 0 | qT], rows 0:50/50:100
    qw = np.zeros((100, 256), np.float16)
    qw[:EMB, 0:BATCH] = q.T
    qw[EMB:2 * EMB, BATCH:256] = q.T

    # weight prepacking: EW = emb @ W.T + b (function of parameters only)
    EWp = np.zeros((VPAD, OUT), np.float32)
    EWp[:VOCAB] = emb @ W.T + b[None, :]

    flat = (np.arange(BATCH, dtype=np.int64)[:, None] * VPAD
            + k.astype(np.int64)).ravel()
    C = np.bincount(flat, minlength=BATCH * VPAD).reshape(BATCH, VPAD)
    assert C.max() <= 255, "count histogram overflows uint8 transport"

    in_maps = []
    for core in range(N_CORES):
        v0 = core * VSH
        blocks = embT[:, v0:v0 + VSH].reshape(EMB, CSH, 128)
        e2 = np.zeros((100, NPAIR, 128), np.float16)
        e2[:EMB] = blocks[:, 0::2, :]
        e2[EMB:2 * EMB] = blocks[:, 1::2, :]
        e2 = np.ascontiguousarray(e2.reshape(100, NPAIR * 128))

        # st9 per quad: cols 2j+o = EW[chunk 4q+j, p, o]; col 8 = 1
        ew_blocks = EWp[v0:v0 + VSH].reshape(CSH, 128, OUT)  # [50, 128, 2]
        st = np.zeros((128, NQUAD, 9), np.float32)
        for quad in range(NQUAD):
            for j in range(4):
                ch = 4 * quad + j
                if ch < CSH:
                    st[:, quad, 2 * j:2 * j + 2] = ew_blocks[ch]
        st[:, :, 8] = 1.0
        st = np.ascontiguousarray(
            st.reshape(128, NQUAD * 9).astype(np.float16))

        ct = np.ascontiguousarray(
            C[:, v0:v0 + VSH].reshape(BATCH, CSH, 128)
            .transpose(2, 1, 0).reshape(128, CSH * BATCH)
            .astype(np.uint8))
        in_maps.append({"et": e2, "qw": qw, "st": st, "ct": ct})
    return in_maps


def _run_device(in_maps, **kwargs):
    from concourse.bass_utils import run_bass_kernel_spmd

    if "nc" not in _CACHE:
        _CACHE["nc"] = _build_nc()
    return run_bass_kernel_spmd(_CACHE["nc"], in_maps,
                                core_ids=list(range(N_CORES)), **kwargs)


def _unshard(res):
    P = np.zeros((9, 512), np.float64)
    for i in range(N_CORES):
        o = res.results[i]["o"].astype(np.float64)
        P += o[0:9] + o[9:18]
    numer = np.zeros((OUT, BATCH), np.float64)
    denom = np.zeros(BATCH, np.float64)
    for j in range(4):
        numer += P[2 * j:2 * j + 2, j * BATCH:(j + 1) * BATCH]
        denom += P[8, j * BATCH:(j + 1) * BATCH]
    out = (numer / denom[None, :]).T
    return np.ascontiguousarray(out, dtype=np.float32)


def kernel(q, k, embeddings, W, b, **_unused):
    in_maps = _prep_inputs(q, k, embeddings, W, b)
    res = _run_device(in_maps)
    return _unshard(res)


# revision 14
# speedup vs baseline: 1.0705x; 1.0705x over previous
"""Trainium2 Bass kernel for nn_AttentionSimple (sparse_attention, 8 cores).

Reference (per batch row b):
    e      = embeddings[k[b]]              # [S, E] gather
    scores = q[b] . e[s]                   # [S]
    attn   = softmax(scores); ctx = sum_s attn[s] * e[s]
    out    = ctx @ W.T + b                 # [B, 2]

Algorithm: count-weighted vocab-space softmax — no per-token gathers.
Scores depend on s only through v = k[b, s], so group softmax terms by
vocabulary id:
    c[b, v]  = |{s : k[b, s] = v}|         (histogram of k, built on host)
    l[b, v]  = q[b] . embeddings[v]        (dense PE matmul)
    A        = c * exp(l - 30)             (global bias keeps exp in fp16;
                                            the ratio is shift-invariant)
    out[b]   = (sum_v A[b,v] * EW[v]) / (sum_v A[b,v])
    with EW  = embeddings @ W.T + b        (parameter prepacking, host)

Sharding: padded vocabulary 51200 = 400 chunks of 128, 50 chunks/core.
Each core handles all 128 batch rows for its slice; host sums the 8
partial numerators/denominators and divides.

Pipeline (per core), all wire data fp16 except uint8 counts:
  - comb: [qw | et] in ONE dram tensor so the first DMA slice delivers
    the mm1 moving operand AND block 0's embeddings together (the tiny
    separate qw DMA otherwise loses descriptor arbitration and gates
    everything).  Embedding pairs packed at rows 0:50 / 50:100.
  - mm1: per pair, ps[128v, 256] = et_pair.T @ qw; 4-pair blocks fill a
    [128, 1024] 2-bank PSUM tile.
  - ACT: le = exp(ps - 30) per quad (512 cols), fp16 out.
  - DVE: le *= counts (uint8, exact) per quad.
  - mm2: acc[9, 512] += st9_quad.T @ le_quad (fp16, f32 PSUM accum);
    st9 = [EW_c0..EW_c3 | ones]; split accumulators accA (quads 0-5,
    shipped mid-kernel) / accB (quads 6-12).
  - Input slices ride the ONE Sync queue interleaved in consumption
    order (et slice before its ct slice) — transfers from all queues
    share the 16 DMA engines, so issue order is the only arrival-order
    control.  st rides the Scalar queue.
  - Warm-up matmuls on broadcast const-APs keep the PE busy from its
    first body cycle (DVFS ramp credit).
"""

import numpy as np

BATCH, SEQ, EMB, VOCAB, OUT = 128, 8192, 50, 50000, 2
N_CORES = 8
CSH = 50                         # vocab chunks per core
NCHUNK = CSH * N_CORES           # 400
VPAD = NCHUNK * 128              # 51200
VSH = CSH * 128                  # 6400
NPAIR = CSH // 2                 # 25
NQUAD = 13                       # 12 full + 1 half
QOFF = 256                       # qw columns at the head of comb
EXP_BIAS = -30.0
NWARM = 6

_CACHE = {}


def _build_nc():
    from contextlib import ExitStack

    import concourse.mybir as mybir
    import concourse.tile as tile
    from concourse import bacc

    f32 = mybir.dt.float32
    f16 = mybir.dt.float16
    bf16 = mybir.dt.bfloat16
    u8 = mybir.dt.uint8
    nc = bacc.Bacc("TRN2", target_bir_lowering=False, debug=False,
                   num_devices=N_CORES)

    comb_d = nc.dram_tensor("comb", [100, QOFF + NPAIR * 128], f16,
                            kind="ExternalInput")
    st_d = nc.dram_tensor("st", [128, NQUAD * 9], f16, kind="ExternalInput")
    ct_d = nc.dram_tensor("ct", [128, VSH], u8, kind="ExternalInput")
    o_d = nc.dram_tensor("o", [18, 512], f32, kind="ExternalOutput")

    with tile.TileContext(nc) as tc, ExitStack() as ctx:
        const_p = ctx.enter_context(tc.tile_pool(name="const", bufs=1))
        ps_p = ctx.enter_context(tc.tile_pool(name="ps", bufs=3, space="PSUM"))
        acc_p = ctx.enter_context(tc.tile_pool(name="acc", bufs=1,
                                               space="PSUM"))
        le_p = ctx.enter_context(tc.tile_pool(name="le", bufs=6))

        # Two accumulators: accA (quads 0-5) finalizes mid-kernel so its
        # copy + output DMA overlap the remaining blocks; accB takes the
        # rest.  Warm-up matmuls write into accB's bank (start=True on the
        # first real accB matmul resets it), keeping PSUM within 8 banks.
        accA = acc_p.tile([9, 512], f32, tag="accA")
        accBf = acc_p.tile([128, 512], f32, tag="accB")
        accB = accBf[0:9, :]

        # PE warm-up on broadcast const-APs (already memset by the
        # framework preamble): busy from the PE's first body cycle.
        wl = nc.const_aps.tensor(1.0, (128, 128), bf16)
        wr = nc.const_aps.tensor(1.0, (128, 512), bf16)
        for _ in range(NWARM):
            nc.tensor.matmul(accBf[:], lhsT=wl, rhs=wr,
                             start=True, stop=True, skip_group_check=True)

        bias_sb = const_p.tile([128, 1], f32)
        nc.gpsimd.memset(bias_sb[:], EXP_BIAS)

        st_sb = const_p.tile([128, NQUAD * 9], f16)
        nc.scalar.dma_start(st_sb[:], st_d.ap())
        comb_sb = const_p.tile([100, QOFF + NPAIR * 128], f16)
        ct_sb = const_p.tile([128, VSH], u8)
        # One queue, consumption order: each et slice just before the ct
        # slice that follows it in the pipeline.
        for dst, dram, c0, c1 in (
                (comb_sb, comb_d, 0, 512),       # qw + pairs 0-1 (b0)
                (ct_sb, ct_d, 0, 512),           # counts b0
                (comb_sb, comb_d, 512, 1792),    # pairs 2-11 (b1-b2+)
                (ct_sb, ct_d, 512, 2560),        # counts b1-b2
                (comb_sb, comb_d, 1792, 2816),   # pairs 12-19 (b3-b4)
                (ct_sb, ct_d, 2560, 4608),       # counts b3-b4
                (comb_sb, comb_d, 2816, 3456),   # pairs 20-24 (b5-b7)
                (ct_sb, ct_d, 4608, VSH),        # counts b5-b7
        ):
            nc.sync.dma_start(dst[:, c0:c1], dram.ap()[:, c0:c1])
        qw = comb_sb[:, 0:QOFF]

        osbA = const_p.tile([9, 512], f32)
        osbB = const_p.tile([9, 512], f32)

        # Blocks: (pair0, quad0, npair); small head and tail blocks keep
        # the pipeline fill and drain short.
        BLOCKS = [(0, 0, 2), (2, 1, 4), (6, 3, 4), (10, 5, 4),
                  (14, 7, 4), (18, 9, 4), (22, 11, 2), (24, 12, 1)]
        for blk, (pair0, quad0, npair) in enumerate(BLOCKS):
            ncol = 256 * npair
            ps = ps_p.tile([128, 1024], f32, tag="ps")
            for lp in range(npair):
                pair = pair0 + lp
                nc.tensor.matmul(
                    ps[:, lp * 256:(lp + 1) * 256],
                    lhsT=comb_sb[:, QOFF + pair * 128:QOFF + (pair + 1) * 128],
                    rhs=qw,
                    start=True, stop=True,
                )
            nquad = (npair + 1) // 2
            for lq in range(nquad):
                quad = quad0 + lq
                qcol = min(512, ncol - lq * 512)
                le = le_p.tile([128, 512], f16, tag="le")
                nc.scalar.activation(le[:, 0:qcol],
                                     ps[:, lq * 512:lq * 512 + qcol],
                                     mybir.ActivationFunctionType.Exp,
                                     bias=bias_sb[:])
                nc.vector.tensor_mul(
                    le[:, 0:qcol], le[:, 0:qcol],
                    ct_sb[:, quad * 512:quad * 512 + qcol])
                acc = accA if quad < 6 else accB
                nc.tensor.matmul(
                    acc if qcol == 512 else acc[:, 0:qcol],
                    lhsT=st_sb[:, quad * 9:(quad + 1) * 9],
                    rhs=le[:, 0:qcol],
                    start=(quad in (0, 6)), stop=(quad in (5, NQUAD - 1)),
                    skip_group_check=True,
                )
            if blk == 3:
                # accA finalized (quads 0-5): ship it while blocks 4-7 run.
                nc.scalar.copy(osbA[:], accA)
                nc.sync.dma_start(o_d.ap()[0:9, :], osbA[:])

        nc.scalar.copy(osbB[:], accB)
        nc.scalar.dma_start(o_d.ap()[9:18, :], osbB[:])

    nc.finalize()
    return nc


def _prep_inputs(q, k, embeddings, W, b):
    q = np.ascontiguousarray(q, dtype=np.float32)
    emb = np.ascontiguousarray(embeddings, dtype=np.float32)
    W = np.ascontiguousarray(W, dtype=np.float32)
    b = np.ascontiguousarray(b, dtype=np.float32)
    k = np.asarray(k)

    embT = np.zeros((EMB, VPAD), np.float32)
    embT[:, :VOCAB] = emb.T

    # weight prepacking: EW = emb @ W.T + b (function of parameters only)
    EWp = np.zeros((VPAD, OUT), np.float32)
    EWp[:VOCAB] = emb @ W.T + b[None, :]

    flat = (np.arange(BATCH, dtype=np.int64)[:, None] * VPAD
            + k.astype(np.int64)).ravel()
    C = np.bincount(flat, minlength=BATCH * VPAD).reshape(BATCH, VPAD)
    assert C.max() <= 255, "count histogram overflows uint8 transport"

    in_maps = []
    for core in range(N_CORES):
        v0 = core * VSH
        blocks = embT[:, v0:v0 + VSH].reshape(EMB, CSH, 128)
        comb = np.zeros((100, QOFF + NPAIR * 128), np.float16)
        # mm1 moving operand: block-diagonal [qT | 0; 0 | qT]
        comb[:EMB, 0:BATCH] = q.T
        comb[EMB:2 * EMB, BATCH:QOFF] = q.T
        e2 = comb[:, QOFF:].reshape(100, NPAIR, 128)
        e2[:EMB] = blocks[:, 0::2, :]
        e2[EMB:2 * EMB] = blocks[:, 1::2, :]

        # st9 per quad: cols 2j+o = EW[chunk 4q+j, p, o]; col 8 = 1
        ew_blocks = EWp[v0:v0 + VSH].reshape(CSH, 128, OUT)  # [50, 128, 2]
        st = np.zeros((128, NQUAD, 9), np.float32)
        for quad in range(NQUAD):
            for j in range(4):
                ch = 4 * quad + j
                if ch < CSH:
                    st[:, quad, 2 * j:2 * j + 2] = ew_blocks[ch]
        st[:, :, 8] = 1.0
        st = np.ascontiguousarray(
            st.reshape(128, NQUAD * 9).astype(np.float16))

        ct = np.ascontiguousarray(
            C[:, v0:v0 + VSH].reshape(BATCH, CSH, 128)
            .transpose(2, 1, 0).reshape(128, CSH * BATCH)
            .astype(np.uint8))
        in_maps.append({"comb": comb, "st": st, "ct": ct})
    return in_maps


def _run_device(in_maps, **kwargs):
    from concourse.bass_utils import run_bass_kernel_spmd

    if "nc" not in _CACHE:
        _CACHE["nc"] = _build_nc()
    return run_bass_kernel_spmd(_CACHE["nc"], in_maps,
                                core_ids=list(range(N_CORES)), **kwargs)


def _unshard(res):
    P = np.zeros((9, 512), np.float64)
    for i in range(N_CORES):
        o = res.results[i]["o"].astype(np.float64)
        P += o[0:9] + o[9:18]
    numer = np.zeros((OUT, BATCH), np.float64)
    denom = np.zeros(BATCH, np.float64)
    for j in range(4):
        numer += P[2 * j:2 * j + 2, j * BATCH:(j + 1) * BATCH]
        denom += P[8, j * BATCH:(j + 1) * BATCH]
    out = (numer / denom[None, :]).T
    return np.ascontiguousarray(out, dtype=np.float32)


def kernel(q, k, embeddings, W, b, **_unused):
    in_maps = _prep_inputs(q, k, embeddings, W, b)
    res = _run_device(in_maps)
    return _unshard(res)


# revision 17
# speedup vs baseline: 1.1168x; 1.0432x over previous
"""Trainium2 Bass kernel for nn_AttentionSimple (sparse_attention, 8 cores).

Reference (per batch row b):
    e      = embeddings[k[b]]              # [S, E] gather
    scores = q[b] . e[s]                   # [S]
    attn   = softmax(scores); ctx = sum_s attn[s] * e[s]
    out    = ctx @ W.T + b                 # [B, 2]

Algorithm: count-weighted vocab-space softmax — no per-token gathers.
Scores depend on s only through v = k[b, s], so group softmax terms by
vocabulary id:
    c[b, v]  = |{s : k[b, s] = v}|         (histogram of k, built on host)
    l[b, v]  = q[b] . embeddings[v]        (dense PE matmul)
    A        = c * exp(l - 30)             (global bias keeps exp in fp16;
                                            the ratio is shift-invariant)
    out[b]   = (sum_v A[b,v] * EW[v]) / (sum_v A[b,v])
    with EW  = embeddings @ W.T + b        (parameter prepacking, host)

Sharding: padded vocabulary 51200 = 400 chunks of 128, 50 chunks/core.
Each core handles all 128 batch rows for its slice; host sums the 8
partial numerators/denominators and divides.

Pipeline (per core), all wire data fp16 except uint8 counts:
  - comb: [qw | et] in ONE dram tensor so the first DMA slice delivers
    the mm1 moving operand AND block 0's embeddings together (the tiny
    separate qw DMA otherwise loses descriptor arbitration and gates
    everything).  Embedding pairs packed at rows 0:50 / 50:100.
  - mm1: per pair, ps[128v, 256] = et_pair.T @ qw; 4-pair blocks fill a
    [128, 1024] 2-bank PSUM tile.
  - ACT: le = exp(ps - 30) per quad (512 cols), fp16 out.
  - DVE: le *= counts (uint8, exact) per quad.
  - mm2: acc[9, 512] += st9_quad.T @ le_quad (fp16, f32 PSUM accum);
    st9 = [EW_c0..EW_c3 | ones]; split accumulators accA (quads 0-5,
    shipped mid-kernel) / accB (quads 6-12).
  - Input slices ride the ONE Sync queue interleaved in consumption
    order (et slice before its ct slice) — transfers from all queues
    share the 16 DMA engines, so issue order is the only arrival-order
    control.  st rides the Scalar queue.
  - Warm-up matmuls on broadcast const-APs keep the PE busy from its
    first body cycle (DVFS ramp credit).
"""

import numpy as np

BATCH, SEQ, EMB, VOCAB, OUT = 128, 8192, 50, 50000, 2
N_CORES = 8
CSH = 50                         # vocab chunks per core
NCHUNK = CSH * N_CORES           # 400
VPAD = NCHUNK * 128              # 51200
VSH = CSH * 128                  # 6400
NPAIR = CSH // 2                 # 25
NQUAD = 13                       # 12 full + 1 half
QOFF = 256                       # qw columns at the head of comb
EXP_BIAS = -30.0
NWARM = 6

_CACHE = {}


def _build_nc():
    from contextlib import ExitStack

    import concourse.mybir as mybir
    import concourse.tile as tile
    from concourse import bacc

    f32 = mybir.dt.float32
    f16 = mybir.dt.float16
    bf16 = mybir.dt.bfloat16
    u8 = mybir.dt.uint8
    nc = bacc.Bacc("TRN2", target_bir_lowering=False, debug=False,
                   num_devices=N_CORES)

    comb_d = nc.dram_tensor("comb", [100, QOFF + NPAIR * 128], f16,
                            kind="ExternalInput")
    st_d = nc.dram_tensor("st", [128, NQUAD * 9], f16, kind="ExternalInput")
    ct_d = nc.dram_tensor("ct", [128, VSH], u8, kind="ExternalInput")
    o_d = nc.dram_tensor("o", [18, 512], f32, kind="ExternalOutput")

    with tile.TileContext(nc) as tc, ExitStack() as ctx:
        const_p = ctx.enter_context(tc.tile_pool(name="const", bufs=1))
        ps_p = ctx.enter_context(tc.tile_pool(name="ps", bufs=2, space="PSUM"))
        acc_p = ctx.enter_context(tc.tile_pool(name="acc", bufs=1,
                                               space="PSUM"))
        le_p = ctx.enter_context(tc.tile_pool(name="le", bufs=6))

        # Two accumulators: accA (quads 0-5) finalizes mid-kernel so its
        # copy + output DMA overlap the remaining blocks; accB takes the
        # rest.  Warm-up matmuls write into accB's bank (start=True on the
        # first real accB matmul resets it), keeping PSUM within 8 banks.
        accA = acc_p.tile([9, 512], f32, tag="accA")
        accBf = acc_p.tile([128, 512], f32, tag="accB")
        accB = accBf[0:9, :]

        # PE warm-up on broadcast const-APs (already memset by the
        # framework preamble): busy from the PE's first body cycle.
        wl = nc.const_aps.tensor(1.0, (128, 128), bf16)
        wr = nc.const_aps.tensor(1.0, (128, 512), bf16)
        for _ in range(NWARM):
            nc.tensor.matmul(accBf[:], lhsT=wl, rhs=wr,
                             start=True, stop=True, skip_group_check=True)

        bias_sb = const_p.tile([128, 1], f32)
        nc.gpsimd.memset(bias_sb[:], EXP_BIAS)

        st_sb = const_p.tile([128, NQUAD * 9], f16)
        nc.scalar.dma_start(st_sb[:], st_d.ap())
        comb_sb = const_p.tile([100, QOFF + NPAIR * 128], f16)
        ct_sb = const_p.tile([128, VSH], u8)
        # One queue, consumption order: each et slice just before the ct
        # slice that follows it in the pipeline.
        for dst, dram, c0, c1 in (
                (comb_sb, comb_d, 0, 1024),      # qw + pairs 0-5 (b0-b1)
                (ct_sb, ct_d, 0, 1536),          # counts q0-q2
                (comb_sb, comb_d, 1024, 2048),   # pairs 6-13 (b2-b3)
                (ct_sb, ct_d, 1536, 3584),       # counts q3-q6
                (comb_sb, comb_d, 2048, 3456),   # pairs 14-24 (b4-b7)
                (ct_sb, ct_d, 3584, VSH),        # counts q7-q12
        ):
            nc.sync.dma_start(dst[:, c0:c1], dram.ap()[:, c0:c1])
        qw = comb_sb[:, 0:QOFF]

        osbA = const_p.tile([9, 512], f32)
        osbB = const_p.tile([9, 512], f32)

        # Blocks: (pair0, quad0, npair); small head and tail blocks keep
        # the pipeline fill and drain short.
        BLOCKS = [(0, 0, 2), (2, 1, 4), (6, 3, 4), (10, 5, 4),
                  (14, 7, 4), (18, 9, 4), (22, 11, 2), (24, 12, 1)]
        for blk, (pair0, quad0, npair) in enumerate(BLOCKS):
            ncol = 256 * npair
            ps = ps_p.tile([128, 1024], f32, tag="ps")
            for lp in range(npair):
                pair = pair0 + lp
                nc.tensor.matmul(
                    ps[:, lp * 256:(lp + 1) * 256],
                    lhsT=comb_sb[:, QOFF + pair * 128:QOFF + (pair + 1) * 128],
                    rhs=qw,
                    start=True, stop=True,
                )
            nquad = (npair + 1) // 2
            for lq in range(nquad):
                quad = quad0 + lq
                qcol = min(512, ncol - lq * 512)
                le = le_p.tile([128, 512], f16, tag="le")
                nc.scalar.activation(le[:, 0:qcol],
                                     ps[:, lq * 512:lq * 512 + qcol],
                                     mybir.ActivationFunctionType.Exp,
                                     bias=bias_sb[:])
                nc.vector.tensor_mul(
                    le[:, 0:qcol], le[:, 0:qcol],
                    ct_sb[:, quad * 512:quad * 512 + qcol])
                acc = accA if quad < 6 else accB
                nc.tensor.matmul(
                    acc if qcol == 512 else acc[:, 0:qcol],
                    lhsT=st_sb[:, quad * 9:(quad + 1) * 9],
                    rhs=le[:, 0:qcol],
                    start=(quad in (0, 6)), stop=(quad in (5, NQUAD - 1)),
                    skip_group_check=True,
                )
            if blk == 3:
                # accA finalized (quads 0-5): ship it while blocks 4-7 run.
                # The copy rides the Vector queue (idle mid-kernel) so the
                # Scalar queue never interrupts the exp stream.
                nc.vector.tensor_copy(osbA[:], accA)
                nc.sync.dma_start(o_d.ap()[0:9, :], osbA[:])

        nc.vector.tensor_copy(osbB[:], accB)
        nc.sync.dma_start(o_d.ap()[9:18, :], osbB[:])

    nc.finalize()
    return nc


def _prep_inputs(q, k, embeddings, W, b):
    q = np.ascontiguousarray(q, dtype=np.float32)
    emb = np.ascontiguousarray(embeddings, dtype=np.float32)
    W = np.ascontiguousarray(W, dtype=np.float32)
    b = np.ascontiguousarray(b, dtype=np.float32)
    k = np.asarray(k)

    embT = np.zeros((EMB, VPAD), np.float32)
    embT[:, :VOCAB] = emb.T

    # weight prepacking: EW = emb @ W.T + b (function of parameters only)
    EWp = np.zeros((VPAD, OUT), np.float32)
    EWp[:VOCAB] = emb @ W.T + b[None, :]

    flat = (np.arange(BATCH, dtype=np.int64)[:, None] * VPAD
            + k.astype(np.int64)).ravel()
    C = np.bincount(flat, minlength=BATCH * VPAD).reshape(BATCH, VPAD)
    assert C.max() <= 255, "count histogram overflows uint8 transport"

    in_maps = []
    for core in range(N_CORES):
        v0 = core * VSH
        blocks = embT[:, v0:v0 + VSH].reshape(EMB, CSH, 128)
        comb = np.zeros((100, QOFF + NPAIR * 128), np.float16)
        # mm1 moving operand: block-diagonal [qT | 0; 0 | qT]
        comb[:EMB, 0:BATCH] = q.T
        comb[EMB:2 * EMB, BATCH:QOFF] = q.T
        e2 = comb[:, QOFF:].reshape(100, NPAIR, 128)
        e2[:EMB] = blocks[:, 0::2, :]
        e2[EMB:2 * EMB] = blocks[:, 1::2, :]

        # st9 per quad: cols 2j+o = EW[chunk 4q+j, p, o]; col 8 = 1
        ew_blocks = EWp[v0:v0 + VSH].reshape(CSH, 128, OUT)  # [50, 128, 2]
        st = np.zeros((128, NQUAD, 9), np.float32)
        for quad in range(NQUAD):
            for j in range(4):
                ch = 4 * quad + j
                if ch < CSH:
                    st[:, quad, 2 * j:2 * j + 2] = ew_blocks[ch]
        st[:, :, 8] = 1.0
        st = np.ascontiguousarray(
            st.reshape(128, NQUAD * 9).astype(np.float16))

        ct = np.ascontiguousarray(
            C[:, v0:v0 + VSH].reshape(BATCH, CSH, 128)
            .transpose(2, 1, 0).reshape(128, CSH * BATCH)
            .astype(np.uint8))
        in_maps.append({"comb": comb, "st": st, "ct": ct})
    return in_maps


def _run_device(in_maps, **kwargs):
    from concourse.bass_utils import run_bass_kernel_spmd

    if "nc" not in _CACHE:
        _CACHE["nc"] = _build_nc()
    return run_bass_kernel_spmd(_CACHE["nc"], in_maps,
                                core_ids=list(range(N_CORES)), **kwargs)


def _unshard(res):
    P = np.zeros((9, 512), np.float64)
    for i in range(N_CORES):
        o = res.results[i]["o"].astype(np.float64)
        P += o[0:9] + o[9:18]
    numer = np.zeros((OUT, BATCH), np.float64)
    denom = np.zeros(BATCH, np.float64)
    for j in range(4):
        numer += P[2 * j:2 * j + 2, j * BATCH:(j + 1) * BATCH]
        denom += P[8, j * BATCH:(j + 1) * BATCH]
    out = (numer / denom[None, :]).T
    return np.ascontiguousarray(out, dtype=np.float32)


def kernel(q, k, embeddings, W, b, **_unused):
    in_maps = _prep_inputs(q, k, embeddings, W, b)
    res = _run_device(in_maps)
    return _unshard(res)
